# revision 61
# baseline (speedup 1.0000x reference)
"""Sparsemax attention (B=8, L=S=1024, H=16, E=D=64) on 8 trn2 NeuronCores.

Sharding: core b handles batch b (all 16 heads).

Per (b,h) pair:
  z = (Q/8) @ K^T   via two stacked bf16 matmuls (exact split product):
        z = [Qh;Ql]^T [Kh;Kh]  +  [Qh;Ql]^T [Kl;Kl]
  tau per row (exact sparsemax threshold): per-128-chunk top-8
        (vector.max) -> 64 candidates -> top16 via max/match_replace/max
        -> log-shift cumsum -> tau = max_j (cumsum_j - 1)/j -> seg reduce
  z^T - tau via two bf16 matmuls with the bias folded into a K=66
        contraction:  [Kh;Kl]^T [Qh;Qh]  +  [Kh;1;1]^T [Ql;-tau_h;-tau_l]
  A^T = relu(.) evicted to fp16 by ACT (a few halves on DVE for balance)
  O^T = V^T A^T in two fp16 terms (V split hi/lo), accumulated in PSUM;
        written to DRAM as [D, L] contiguously; the host untransposes.
"""

import numpy as np

N_CORES = 8
NP_PAIRS = 16  # heads per core
L = 1024
S = 1024
E = 64
D = 64
LT = L // 128
SC = S // 128

_CACHE = {}


def _apply_tile_drain_patch():
    import concourse.tile as tile
    import concourse.mybir as mybir
    from concourse.vector_clock import ScopedClock

    if getattr(tile.TileContext, "_drain_patched", False):
        return

    def _patched(self, tick_clock, wait_clock):
        nc = self.nc
        collector = nc.sync.nop()
        wait_clock.add_sem_waits(
            collector.ins, ScopedClock({None: tick_clock.global_clock})
        )
        waits = list(collector.ins.sync_info.on_wait)
        if len(waits) > 1:
            collector.ins.sync_info.on_wait = waits[:1]
            for w in waits[1:]:
                extra = nc.sync.nop()
                extra.ins.sync_info = mybir.SyncInfo(on_wait=[w], on_update=[])
        nc.sync.drain()
        nc.all_engine_barrier()
        assert self.sems is not None
        popped = nc._tile_sem_poison_stack.pop()
        assert popped is self._sem_poison
        nc.clear_and_free_semaphores(list(self.sems.allocated().values()))
        nc.all_engine_barrier()

    tile.TileContext._drain_and_barrier = _patched
    tile.TileContext._drain_patched = True


def _split_excess_waits(nc, limit=1):
    """walrus here rejects >limit sync-waits per instruction; move excess
    waits onto same-engine NOPs placed just before the instruction."""
    import concourse.mybir as mybir

    cnt = 0
    for bb in nc.main_func.blocks:
        il = list(bb.instructions)
        out = []
        changed = False
        for ins in il:
            si = getattr(ins, "sync_info", None)
            ow = list(si.on_wait) if (si is not None and si.on_wait) else []
            if len(ow) > limit:
                changed = True
                for w in ow[: len(ow) - limit]:
                    cnt += 1
                    nop = mybir.InstNoOp(name=f"WSPLIT-{cnt}", ins=[], outs=[])
                    nop.engine = ins.engine
                    nop.sync_info = mybir.SyncInfo(on_wait=[w], on_update=[])
                    nc.register_instruction(nop, overwrite=True)
                    out.append(nop)
                si.on_wait = ow[len(ow) - limit :]
            out.append(ins)
        if changed:
            bb.instructions = out
    return cnt


def build_program(n_pairs=NP_PAIRS):
    import concourse.bass as bass
    import concourse.mybir as mybir
    import concourse.tile as tile
    from concourse.bass import ts, ds

    _apply_tile_drain_patch()

    f32 = mybir.dt.float32
    bf16 = mybir.dt.bfloat16
    f16 = mybir.dt.float16
    nc = bass.Bass()

    # bf16 stacked score operands
    qhh_d = nc.dram_tensor("qhh", [n_pairs, 128, L], bf16, kind="ExternalInput")
    qll_d = nc.dram_tensor("qll", [n_pairs, 128, L], bf16, kind="ExternalInput")
    khh_d = nc.dram_tensor("khh", [n_pairs, 128, S], bf16, kind="ExternalInput")
    kst_d = nc.dram_tensor("kst", [n_pairs, 128, S], bf16, kind="ExternalInput")
    kh66_d = nc.dram_tensor("kh66", [n_pairs, 66, S], bf16, kind="ExternalInput")
    qlt_d = nc.dram_tensor("qlt", [n_pairs, 66, L], bf16, kind="ExternalInput")
    vh_d = nc.dram_tensor("vh", [n_pairs, S, D], f16, kind="ExternalInput")
    vl_d = nc.dram_tensor("vl", [n_pairs, S, D], f16, kind="ExternalInput")
    recip_d = nc.dram_tensor("recip", [128, 128], f32, kind="ExternalInput")
    nrecip_d = nc.dram_tensor("nrecip", [128, 128], f32, kind="ExternalInput")
    # output stored transposed per pair: [D, L]; host untransposes
    out = nc.dram_tensor("out", [n_pairs, D, L], f32, kind="ExternalOutput")

    with tile.TileContext(nc) as tc:
        with (
            tc.tile_pool(name="singles", bufs=1) as singles,
            tc.tile_pool(name="weights", bufs=3) as weights,
            tc.tile_pool(name="vt_pool", bufs=3) as vt_pool,
            tc.tile_pool(name="zs_pool", bufs=4) as zs_pool,
            tc.tile_pool(name="at_pool", bufs=4) as at_pool,
            tc.tile_pool(name="cand_pool", bufs=4) as cand_pool,
            tc.tile_pool(name="tau_pool", bufs=2) as tau_pool,
            tc.tile_pool(name="o_pool", bufs=2) as o_pool,
            tc.tile_pool(name="zpsum", bufs=2, space="PSUM") as zpsum,
            tc.tile_pool(name="ztpsum", bufs=2, space="PSUM") as ztpsum,
            tc.tile_pool(name="otpsum", bufs=1, space="PSUM") as otpsum,
        ):
            recipt = singles.tile([128, 128], f32)
            nrecipt = singles.tile([128, 128], f32)
            nc.sync.dma_start(out=recipt[:], in_=recip_d[:])
            nc.sync.dma_start(out=nrecipt[:], in_=nrecip_d[:])
            recip_v = recipt.rearrange("p (s r) -> p s r", r=16)
            nrecip_v = nrecipt.rearrange("p (s r) -> p s r", r=16)

            add = mybir.AluOpType.add
            mult = mybir.AluOpType.mult
            sub = mybir.AluOpType.subtract

            for g in range(n_pairs):
                qhh = weights.tile([128, L], bf16, tag="qhh")
                qll = weights.tile([128, L], bf16, tag="qll")
                khh = weights.tile([128, S], bf16, tag="khh")
                kst = weights.tile([128, S], bf16, tag="kst")
                kh66 = weights.tile([66, S], bf16, tag="kh66")
                qlt = weights.tile([66, L], bf16, tag="qlt")
                vh = vt_pool.tile([128, SC, D], f16, tag="vh")
                vl = vt_pool.tile([128, SC, D], f16, tag="vl")
                nc.sync.dma_start(out=qhh[:], in_=qhh_d[g])
                nc.sync.dma_start(out=qll[:], in_=qll_d[g])
                nc.sync.dma_start(out=khh[:], in_=khh_d[g])
                nc.sync.dma_start(out=kst[:], in_=kst_d[g])
                nc.sync.dma_start(out=kh66[:], in_=kh66_d[g])
                nc.sync.dma_start(out=qlt[:, :], in_=qlt_d[g])
                nc.sync.dma_start(
                    out=vh[:], in_=vh_d[g].rearrange("(c p) d -> p c d", p=128)
                )
                nc.sync.dma_start(
                    out=vl[:], in_=vl_d[g].rearrange("(c p) d -> p c d", p=128)
                )

                csA = tau_pool.tile([128, LT, 24], f32, tag="csA")
                csB = tau_pool.tile([128, LT, 24], f32, tag="csB")
                negtau = tau_pool.tile([128, LT], f32, tag="negtau")
                nthl = tau_pool.tile([128, 32], bf16, tag="nthl")
                wthl = tau_pool.tile([128, 32], bf16, tag="wthl")
                nc.gpsimd.memset(csA[:, :, 0:8], 0.0)
                nc.gpsimd.memset(csB[:, :, 0:8], 0.0)
                nc.gpsimd.memset(nthl[:, 2 * LT : 32], 0.0)

                # --- z pass (3-term bf16 split) + candidates ---
                # z ~= [Qh;Qh]^T kst  +  Ql^T Kh   (drop Ql Kl ~ 2^-18)
                # the Ql Kh term runs as row-packed K=64 matmuls: two
                # l-tiles concurrently in array row-groups 0 and 64.
                zps = {}
                for t in range(LT):
                    zp = zpsum.tile([128, S], f32, tag="scores")
                    zps[t] = zp
                    for h in range(2):
                        nc.tensor.matmul(
                            zp[:, ds(512 * h, 512)],
                            qhh[:, ts(t, 128)],
                            kst[:, ds(512 * h, 512)],
                            start=True,
                            stop=False,
                        )
                    if t % 2 == 1:
                        for h in range(2):
                            nc.tensor.matmul(
                                zps[t - 1][:, ds(512 * h, 512)],
                                qll[0:64, ts(t - 1, 128)],
                                khh[0:64, ds(512 * h, 512)],
                                start=False,
                                stop=True,
                            )
                            nc.tensor.matmul(
                                zps[t][:, ds(512 * h, 512)],
                                qll[64:128, ts(t, 128)],
                                khh[64:128, ds(512 * h, 512)],
                                start=False,
                                stop=True,
                            )
                    if t % 2 == 0:
                        continue
                    for tt in (t - 1, t):
                        zp = zps.pop(tt)
                        zs = zs_pool.tile([128, S], f32, tag="zs")
                        nc.scalar.copy(zs[:], zp[:])
                        cand = cand_pool.tile([128, 64], f32, tag="cand")
                        for c in range(SC):
                            nc.vector.max(cand[:, ts(c, 8)], zs[:, ts(c, 128)])
                        cand2 = cand_pool.tile([128, 64], f32, tag="cand2")
                        nc.vector.max(csA[:, tt, 8:16], cand[:])
                        nc.vector.match_replace(
                            cand2[:], csA[:, tt, 8:16], cand[:], -1e30
                        )
                        nc.vector.max(csA[:, tt, 16:24], cand2[:])

                # --- tau (cumsum-16 + max-identity) on gpsimd; DVE is busy ---
                TT = nc.gpsimd.tensor_tensor
                TT(csB[:, :, 8:24], csA[:, :, 8:24], csA[:, :, 7:23], op=add)
                TT(csA[:, :, 8:24], csB[:, :, 8:24], csB[:, :, 6:22], op=add)
                TT(csB[:, :, 8:24], csA[:, :, 8:24], csA[:, :, 4:20], op=add)
                TT(csA[:, :, 8:24], csB[:, :, 8:24], csB[:, :, 0:16], op=add)
                mbuf = tau_pool.tile([128, LT, 16], f32, tag="mbuf")
                stat = tau_pool.tile([128, LT, 16], f32, tag="stat")
                TT(mbuf[:], csA[:, :, 8:24], nrecip_v[:], op=mult)
                TT(stat[:], mbuf[:], recip_v[:], op=add)
                TT = nc.vector.tensor_tensor
                nc.vector.tensor_reduce(
                    negtau[:],
                    stat[:],
                    axis=mybir.AxisListType.X,
                    op=mybir.AluOpType.min,
                )
                # bf16 hi/lo split of -tau: nthl cols 0:8 = -tau_h, 8:16 = -tau_l
                nc.vector.tensor_copy(nthl[:, 0:LT], negtau[:])
                TT(nthl[:, LT : 2 * LT], negtau[:], nthl[:, 0:LT], op=sub)
                # scatter into qlt rows 64/65 via 32x32 stream transpose:
                # wthl[32P+a, b] = nthl[32P+b, a]
                nc.vector.transpose(wthl[:], nthl[:])
                q64 = qlt[64:65, :].rearrange("o (t Pq b) -> o Pq t b", Pq=4, b=32)
                q65 = qlt[65:66, :].rearrange("o (t Pq b) -> o Pq t b", Pq=4, b=32)
                for P in range(4):
                    nc.sync.dma_start(
                        out=q64[:, P], in_=wthl[ds(32 * P, LT), :]
                    )
                    nc.sync.dma_start(
                        out=q65[:, P], in_=wthl[ds(32 * P + LT, LT), :]
                    )

                # --- z^T - tau (bf16 split + K=66 bias), relu->fp16, AV ---
                ot = otpsum.tile([64, L], f32, tag="ot")
                for c in range(SC):
                    ats = []
                    for h in range(2):
                        ztp = ztpsum.tile([128, 512], f32, tag="zt")
                        nc.tensor.matmul(
                            ztp[:],
                            kst[:, ts(c, 128)],
                            qhh[:, ds(512 * h, 512)],
                            start=True,
                            stop=False,
                        )
                        nc.tensor.matmul(
                            ztp[:],
                            kh66[:, ts(c, 128)],
                            qlt[:, ds(512 * h, 512)],
                            start=False,
                            stop=True,
                        )
                        at = at_pool.tile([128, 512], f16, tag="at")
                        if c >= 5 and h == 1:
                            nc.vector.tensor_scalar_max(at[:], ztp[:], 0.0)
                        else:
                            nc.scalar.activation(
                                at[:], ztp[:], mybir.ActivationFunctionType.Relu
                            )
                        ats.append(at)
                    for h in range(2):
                        nc.tensor.matmul(
                            ot[:, ds(512 * h, 512)],
                            vh[:, c, :],
                            ats[h][:],
                            start=(c == 0),
                            stop=False,
                        )
                        nc.tensor.matmul(
                            ot[:, ds(512 * h, 512)],
                            vl[:, c, :],
                            ats[h][:],
                            start=False,
                            stop=(c == SC - 1),
                        )

                # output written transposed [D, L]; contiguous DMA
                ots = o_pool.tile([64, L], f32, tag="ots")
                nc.scalar.copy(ots[:], ot[:])
                nc.sync.dma_start(out=out[g], in_=ots[:])

    n_split = _split_excess_waits(nc)
    if n_split:
        import sys

        print(f"[kernel] split {n_split} excess sync-waits onto NOPs", file=sys.stderr)
    return nc


def _host_prep(queries, keys, values):
    """Build the 8 per-core input maps from full inputs."""
    import ml_dtypes

    bf16 = ml_dtypes.bfloat16
    B, Lq, H, Ee = queries.shape
    scale = np.float32(1.0 / np.sqrt(np.float32(Ee)))
    jj = np.arange(128, dtype=np.float32)
    recip = np.broadcast_to(1.0 / ((jj % 16) + 1.0), (128, 128)).astype(np.float32)
    nrecip = (-recip).astype(np.float32)

    in_maps = []
    for b in range(B):
        qt = (queries[b].astype(np.float32) * scale).transpose(1, 2, 0)  # [H,E,L]
        kt = keys[b].astype(np.float32).transpose(1, 2, 0)  # [H,E,S]
        vv = values[b].astype(np.float32).transpose(1, 0, 2)  # [H,S,D]

        qh = qt.astype(bf16)
        ql = (qt - qh.astype(np.float32)).astype(bf16)
        kh = kt.astype(bf16)
        kl = (kt - kh.astype(np.float32)).astype(bf16)

        qhh = np.concatenate([qh, qh], axis=1)  # [H,128,L]
        qll = np.concatenate([ql, ql], axis=1)
        khh = np.concatenate([kh, kh], axis=1)
        kst = np.concatenate([kh, kl], axis=1)
        ones_row = np.ones((H, 2, S), dtype=bf16)
        kh66 = np.concatenate([kh, ones_row], axis=1)  # [H,66,S]
        qlt = np.concatenate([ql, np.zeros((H, 2, Lq), dtype=bf16)], axis=1)

        vhf = vv.astype(np.float16)
        vlf = (vv - vhf.astype(np.float32)).astype(np.float16)

        in_maps.append(
            {
                "qhh": np.ascontiguousarray(qhh),
                "qll": np.ascontiguousarray(qll),
                "khh": np.ascontiguousarray(khh),
                "kst": np.ascontiguousarray(kst),
                "kh66": np.ascontiguousarray(kh66),
                "qlt": np.ascontiguousarray(qlt),
                "vh": np.ascontiguousarray(vhf),
                "vl": np.ascontiguousarray(vlf),
                "recip": recip.copy(),
                "nrecip": nrecip.copy(),
            }
        )
    return in_maps


def kernel(queries, keys, values):
    from concourse.bass_utils import run_bass_kernel_spmd

    queries = np.asarray(queries)
    keys = np.asarray(keys)
    values = np.asarray(values)
    B, Lq, H, _ = queries.shape

    if "nc" not in _CACHE:
        _CACHE["nc"] = build_program(NP_PAIRS)
    nc = _CACHE["nc"]

    in_maps = _host_prep(queries, keys, values)
    res = run_bass_kernel_spmd(nc, in_maps, list(range(N_CORES)))

    outp = np.empty((B, Lq, H, D), dtype=np.float32)
    for b in range(B):
        o = res.results[b]["out"]  # [H, D, L]
        outp[b] = o.transpose(2, 0, 1)
    return outp


# revision 62
# speedup vs baseline: 1.0404x; 1.0404x over previous
"""Sparsemax attention (B=8, L=S=1024, H=16, E=D=64) on 8 trn2 NeuronCores.

Sharding: core b handles batch b (all 16 heads).

Per (b,h) pair:
  z = (Q/8) @ K^T   via two stacked bf16 matmuls (exact split product):
        z = [Qh;Ql]^T [Kh;Kh]  +  [Qh;Ql]^T [Kl;Kl]
  tau per row (exact sparsemax threshold): per-128-chunk top-8
        (vector.max) -> 64 candidates -> top16 via max/match_replace/max
        -> log-shift cumsum -> tau = max_j (cumsum_j - 1)/j -> seg reduce
  z^T - tau via two bf16 matmuls with the bias folded into a K=66
        contraction:  [Kh;Kl]^T [Qh;Qh]  +  [Kh;1;1]^T [Ql;-tau_h;-tau_l]
  A^T = relu(.) evicted to fp16 by ACT (a few halves on DVE for balance)
  O^T = V^T A^T in two fp16 terms (V split hi/lo), accumulated in PSUM;
        written to DRAM as [D, L] contiguously; the host untransposes.
"""

import numpy as np

N_CORES = 8
NP_PAIRS = 16  # heads per core
L = 1024
S = 1024
E = 64
D = 64
LT = L // 128
SC = S // 128

_CACHE = {}


def _apply_tile_drain_patch():
    import concourse.tile as tile
    import concourse.mybir as mybir
    from concourse.vector_clock import ScopedClock

    if getattr(tile.TileContext, "_drain_patched", False):
        return

    def _patched(self, tick_clock, wait_clock):
        nc = self.nc
        collector = nc.sync.nop()
        wait_clock.add_sem_waits(
            collector.ins, ScopedClock({None: tick_clock.global_clock})
        )
        waits = list(collector.ins.sync_info.on_wait)
        if len(waits) > 1:
            collector.ins.sync_info.on_wait = waits[:1]
            for w in waits[1:]:
                extra = nc.sync.nop()
                extra.ins.sync_info = mybir.SyncInfo(on_wait=[w], on_update=[])
        nc.sync.drain()
        nc.all_engine_barrier()
        assert self.sems is not None
        popped = nc._tile_sem_poison_stack.pop()
        assert popped is self._sem_poison
        nc.clear_and_free_semaphores(list(self.sems.allocated().values()))
        nc.all_engine_barrier()

    tile.TileContext._drain_and_barrier = _patched
    tile.TileContext._drain_patched = True


def _split_excess_waits(nc, limit=1):
    """walrus here rejects >limit sync-waits per instruction; move excess
    waits onto same-engine NOPs placed just before the instruction."""
    import concourse.mybir as mybir

    cnt = 0
    for bb in nc.main_func.blocks:
        il = list(bb.instructions)
        out = []
        changed = False
        for ins in il:
            si = getattr(ins, "sync_info", None)
            ow = list(si.on_wait) if (si is not None and si.on_wait) else []
            if len(ow) > limit:
                changed = True
                for w in ow[: len(ow) - limit]:
                    cnt += 1
                    nop = mybir.InstNoOp(name=f"WSPLIT-{cnt}", ins=[], outs=[])
                    nop.engine = ins.engine
                    nop.sync_info = mybir.SyncInfo(on_wait=[w], on_update=[])
                    nc.register_instruction(nop, overwrite=True)
                    out.append(nop)
                si.on_wait = ow[len(ow) - limit :]
            out.append(ins)
        if changed:
            bb.instructions = out
    return cnt


def build_program(n_pairs=NP_PAIRS):
    import concourse.bass as bass
    import concourse.mybir as mybir
    import concourse.tile as tile
    from concourse.bass import ts, ds

    _apply_tile_drain_patch()

    f32 = mybir.dt.float32
    bf16 = mybir.dt.bfloat16
    f16 = mybir.dt.float16
    nc = bass.Bass()

    # bf16 stacked score operands
    qhh_d = nc.dram_tensor("qhh", [n_pairs, 128, L], bf16, kind="ExternalInput")
    qll_d = nc.dram_tensor("qll", [n_pairs, 128, L], bf16, kind="ExternalInput")
    khh_d = nc.dram_tensor("khh", [n_pairs, 128, S], bf16, kind="ExternalInput")
    kst_d = nc.dram_tensor("kst", [n_pairs, 128, S], bf16, kind="ExternalInput")
    kh66_d = nc.dram_tensor("kh66", [n_pairs, 66, S], bf16, kind="ExternalInput")
    qlt_d = nc.dram_tensor("qlt", [n_pairs, 66, L], bf16, kind="ExternalInput")
    vh_d = nc.dram_tensor("vh", [n_pairs, S, D], f16, kind="ExternalInput")
    vl_d = nc.dram_tensor("vl", [n_pairs, S, D], f16, kind="ExternalInput")
    recip_d = nc.dram_tensor("recip", [128, 128], f32, kind="ExternalInput")
    nrecip_d = nc.dram_tensor("nrecip", [128, 128], f32, kind="ExternalInput")
    # output stored transposed per pair: [D, L]; host untransposes
    out = nc.dram_tensor("out", [n_pairs, D, L], f32, kind="ExternalOutput")

    with tile.TileContext(nc) as tc:
        with (
            tc.tile_pool(name="singles", bufs=1) as singles,
            tc.tile_pool(name="weights", bufs=4) as weights,
            tc.tile_pool(name="vt_pool", bufs=3) as vt_pool,
            tc.tile_pool(name="zs_pool", bufs=6) as zs_pool,
            tc.tile_pool(name="at_pool", bufs=6) as at_pool,
            tc.tile_pool(name="cand_pool", bufs=6) as cand_pool,
            tc.tile_pool(name="tau_pool", bufs=2) as tau_pool,
            tc.tile_pool(name="o_pool", bufs=2) as o_pool,
            tc.tile_pool(name="zpsum", bufs=2, space="PSUM") as zpsum,
            tc.tile_pool(name="ztpsum", bufs=2, space="PSUM") as ztpsum,
            tc.tile_pool(name="otpsum", bufs=1, space="PSUM") as otpsum,
        ):
            recipt = singles.tile([128, 128], f32)
            nrecipt = singles.tile([128, 128], f32)
            nc.sync.dma_start(out=recipt[:], in_=recip_d[:])
            nc.sync.dma_start(out=nrecipt[:], in_=nrecip_d[:])
            recip_v = recipt.rearrange("p (s r) -> p s r", r=16)
            nrecip_v = nrecipt.rearrange("p (s r) -> p s r", r=16)

            add = mybir.AluOpType.add
            mult = mybir.AluOpType.mult
            sub = mybir.AluOpType.subtract

            for g in range(n_pairs):
                qhh = weights.tile([128, L], bf16, tag="qhh")
                qll = weights.tile([128, L], bf16, tag="qll")
                khh = weights.tile([128, S], bf16, tag="khh")
                kst = weights.tile([128, S], bf16, tag="kst")
                kh66 = weights.tile([66, S], bf16, tag="kh66")
                qlt = weights.tile([66, L], bf16, tag="qlt")
                vh = vt_pool.tile([128, SC, D], f16, tag="vh")
                vl = vt_pool.tile([128, SC, D], f16, tag="vl")
                nc.sync.dma_start(out=qhh[:], in_=qhh_d[g])
                nc.sync.dma_start(out=qll[:], in_=qll_d[g])
                nc.sync.dma_start(out=khh[:], in_=khh_d[g])
                nc.sync.dma_start(out=kst[:], in_=kst_d[g])
                nc.sync.dma_start(out=kh66[:], in_=kh66_d[g])
                nc.sync.dma_start(out=qlt[:, :], in_=qlt_d[g])
                nc.sync.dma_start(
                    out=vh[:], in_=vh_d[g].rearrange("(c p) d -> p c d", p=128)
                )
                nc.sync.dma_start(
                    out=vl[:], in_=vl_d[g].rearrange("(c p) d -> p c d", p=128)
                )

                csA = tau_pool.tile([128, LT, 24], f32, tag="csA")
                csB = tau_pool.tile([128, LT, 24], f32, tag="csB")
                negtau = tau_pool.tile([128, LT], f32, tag="negtau")
                nthl = tau_pool.tile([128, 32], bf16, tag="nthl")
                wthl = tau_pool.tile([128, 32], bf16, tag="wthl")
                nc.gpsimd.memset(csA[:, :, 0:8], 0.0)
                nc.gpsimd.memset(csB[:, :, 0:8], 0.0)
                nc.gpsimd.memset(nthl[:, 2 * LT : 32], 0.0)

                # --- z pass (3-term bf16 split) + candidates ---
                # z ~= [Qh;Qh]^T kst  +  Ql^T Kh   (drop Ql Kl ~ 2^-18)
                # the Ql Kh term runs as row-packed K=64 matmuls: two
                # l-tiles concurrently in array row-groups 0 and 64.
                zps = {}
                for t in range(LT):
                    zp = zpsum.tile([128, S], f32, tag="scores")
                    zps[t] = zp
                    for h in range(2):
                        nc.tensor.matmul(
                            zp[:, ds(512 * h, 512)],
                            qhh[:, ts(t, 128)],
                            kst[:, ds(512 * h, 512)],
                            start=True,
                            stop=False,
                        )
                    if t % 2 == 1:
                        for h in range(2):
                            nc.tensor.matmul(
                                zps[t - 1][:, ds(512 * h, 512)],
                                qll[0:64, ts(t - 1, 128)],
                                khh[0:64, ds(512 * h, 512)],
                                start=False,
                                stop=True,
                            )
                            nc.tensor.matmul(
                                zps[t][:, ds(512 * h, 512)],
                                qll[64:128, ts(t, 128)],
                                khh[64:128, ds(512 * h, 512)],
                                start=False,
                                stop=True,
                            )
                    if t % 2 == 0:
                        continue
                    for tt in (t - 1, t):
                        zp = zps.pop(tt)
                        zs = zs_pool.tile([128, S], f32, tag="zs")
                        nc.scalar.copy(zs[:], zp[:])
                        cand = cand_pool.tile([128, 64], f32, tag="cand")
                        for c in range(SC):
                            nc.vector.max(cand[:, ts(c, 8)], zs[:, ts(c, 128)])
                        cand2 = cand_pool.tile([128, 64], f32, tag="cand2")
                        nc.vector.max(csA[:, tt, 8:16], cand[:])
                        nc.vector.match_replace(
                            cand2[:], csA[:, tt, 8:16], cand[:], -1e30
                        )
                        nc.vector.max(csA[:, tt, 16:24], cand2[:])

                # --- tau (cumsum-16 + max-identity) on gpsimd; DVE is busy ---
                TT = nc.gpsimd.tensor_tensor
                TT(csB[:, :, 8:24], csA[:, :, 8:24], csA[:, :, 7:23], op=add)
                TT(csA[:, :, 8:24], csB[:, :, 8:24], csB[:, :, 6:22], op=add)
                TT(csB[:, :, 8:24], csA[:, :, 8:24], csA[:, :, 4:20], op=add)
                TT(csA[:, :, 8:24], csB[:, :, 8:24], csB[:, :, 0:16], op=add)
                mbuf = tau_pool.tile([128, LT, 16], f32, tag="mbuf")
                stat = tau_pool.tile([128, LT, 16], f32, tag="stat")
                TT(mbuf[:], csA[:, :, 8:24], nrecip_v[:], op=mult)
                TT(stat[:], mbuf[:], recip_v[:], op=add)
                TT = nc.vector.tensor_tensor
                nc.vector.tensor_reduce(
                    negtau[:],
                    stat[:],
                    axis=mybir.AxisListType.X,
                    op=mybir.AluOpType.min,
                )
                # bf16 hi/lo split of -tau: nthl cols 0:8 = -tau_h, 8:16 = -tau_l
                nc.vector.tensor_copy(nthl[:, 0:LT], negtau[:])
                TT(nthl[:, LT : 2 * LT], negtau[:], nthl[:, 0:LT], op=sub)
                # scatter into qlt rows 64/65 via 32x32 stream transpose:
                # wthl[32P+a, b] = nthl[32P+b, a]
                nc.vector.transpose(wthl[:], nthl[:])
                q64 = qlt[64:65, :].rearrange("o (t Pq b) -> o Pq t b", Pq=4, b=32)
                q65 = qlt[65:66, :].rearrange("o (t Pq b) -> o Pq t b", Pq=4, b=32)
                for P in range(4):
                    nc.sync.dma_start(
                        out=q64[:, P], in_=wthl[ds(32 * P, LT), :]
                    )
                    nc.sync.dma_start(
                        out=q65[:, P], in_=wthl[ds(32 * P + LT, LT), :]
                    )

                # --- z^T - tau (bf16 split + K=66 bias), relu->fp16, AV ---
                ot = otpsum.tile([64, L], f32, tag="ot")
                for c in range(SC):
                    ats = []
                    for h in range(2):
                        ztp = ztpsum.tile([128, 512], f32, tag="zt")
                        nc.tensor.matmul(
                            ztp[:],
                            kst[:, ts(c, 128)],
                            qhh[:, ds(512 * h, 512)],
                            start=True,
                            stop=False,
                        )
                        nc.tensor.matmul(
                            ztp[:],
                            kh66[:, ts(c, 128)],
                            qlt[:, ds(512 * h, 512)],
                            start=False,
                            stop=True,
                        )
                        at = at_pool.tile([128, 512], f16, tag="at")
                        if c >= 4 and h == 1:
                            nc.vector.tensor_scalar_max(at[:], ztp[:], 0.0)
                        else:
                            nc.scalar.activation(
                                at[:], ztp[:], mybir.ActivationFunctionType.Relu
                            )
                        ats.append(at)
                    for h in range(2):
                        nc.tensor.matmul(
                            ot[:, ds(512 * h, 512)],
                            vh[:, c, :],
                            ats[h][:],
                            start=(c == 0),
                            stop=False,
                        )
                        nc.tensor.matmul(
                            ot[:, ds(512 * h, 512)],
                            vl[:, c, :],
                            ats[h][:],
                            start=False,
                            stop=(c == SC - 1),
                        )

                # output written transposed [D, L]; contiguous DMA
                ots = o_pool.tile([64, L], f32, tag="ots")
                nc.scalar.copy(ots[:], ot[:])
                nc.sync.dma_start(out=out[g], in_=ots[:])

    n_split = _split_excess_waits(nc)
    if n_split:
        import sys

        print(f"[kernel] split {n_split} excess sync-waits onto NOPs", file=sys.stderr)
    return nc


def _host_prep(queries, keys, values):
    """Build the 8 per-core input maps from full inputs."""
    import ml_dtypes

    bf16 = ml_dtypes.bfloat16
    B, Lq, H, Ee = queries.shape
    scale = np.float32(1.0 / np.sqrt(np.float32(Ee)))
    jj = np.arange(128, dtype=np.float32)
    recip = np.broadcast_to(1.0 / ((jj % 16) + 1.0), (128, 128)).astype(np.float32)
    nrecip = (-recip).astype(np.float32)

    in_maps = []
    for b in range(B):
        qt = (queries[b].astype(np.float32) * scale).transpose(1, 2, 0)  # [H,E,L]
        kt = keys[b].astype(np.float32).transpose(1, 2, 0)  # [H,E,S]
        vv = values[b].astype(np.float32).transpose(1, 0, 2)  # [H,S,D]

        qh = qt.astype(bf16)
        ql = (qt - qh.astype(np.float32)).astype(bf16)
        kh = kt.astype(bf16)
        kl = (kt - kh.astype(np.float32)).astype(bf16)

        qhh = np.concatenate([qh, qh], axis=1)  # [H,128,L]
        qll = np.concatenate([ql, ql], axis=1)
        khh = np.concatenate([kh, kh], axis=1)
        kst = np.concatenate([kh, kl], axis=1)
        ones_row = np.ones((H, 2, S), dtype=bf16)
        kh66 = np.concatenate([kh, ones_row], axis=1)  # [H,66,S]
        qlt = np.concatenate([ql, np.zeros((H, 2, Lq), dtype=bf16)], axis=1)

        vhf = vv.astype(np.float16)
        vlf = (vv - vhf.astype(np.float32)).astype(np.float16)

        in_maps.append(
            {
                "qhh": np.ascontiguousarray(qhh),
                "qll": np.ascontiguousarray(qll),
                "khh": np.ascontiguousarray(khh),
                "kst": np.ascontiguousarray(kst),
                "kh66": np.ascontiguousarray(kh66),
                "qlt": np.ascontiguousarray(qlt),
                "vh": np.ascontiguousarray(vhf),
                "vl": np.ascontiguousarray(vlf),
                "recip": recip.copy(),
                "nrecip": nrecip.copy(),
            }
        )
    return in_maps


def kernel(queries, keys, values):
    from concourse.bass_utils import run_bass_kernel_spmd

    queries = np.asarray(queries)
    keys = np.asarray(keys)
    values = np.asarray(values)
    B, Lq, H, _ = queries.shape

    if "nc" not in _CACHE:
        _CACHE["nc"] = build_program(NP_PAIRS)
    nc = _CACHE["nc"]

    in_maps = _host_prep(queries, keys, values)
    res = run_bass_kernel_spmd(nc, in_maps, list(range(N_CORES)))

    outp = np.empty((B, Lq, H, D), dtype=np.float32)
    for b in range(B):
        o = res.results[b]["out"]  # [H, D, L]
        outp[b] = o.transpose(2, 0, 1)
    return outp


# revision 63
# speedup vs baseline: 1.0586x; 1.0175x over previous
"""Sparsemax attention (B=8, L=S=1024, H=16, E=D=64) on 8 trn2 NeuronCores.

Sharding: core b handles batch b (all 16 heads).

Per (b,h) pair:
  z = (Q/8) @ K^T   via two stacked bf16 matmuls (exact split product):
        z = [Qh;Ql]^T [Kh;Kh]  +  [Qh;Ql]^T [Kl;Kl]
  tau per row (exact sparsemax threshold): per-128-chunk top-8
        (vector.max) -> 64 candidates -> top16 via max/match_replace/max
        -> log-shift cumsum -> tau = max_j (cumsum_j - 1)/j -> seg reduce
  z^T - tau via two bf16 matmuls with the bias folded into a K=66
        contraction:  [Kh;Kl]^T [Qh;Qh]  +  [Kh;1;1]^T [Ql;-tau_h;-tau_l]
  A^T = relu(.) evicted to fp16 by ACT (a few halves on DVE for balance)
  O^T = V^T A^T in two fp16 terms (V split hi/lo), accumulated in PSUM;
        written to DRAM as [D, L] contiguously; the host untransposes.
"""

import numpy as np

N_CORES = 8
NP_PAIRS = 16  # heads per core
L = 1024
S = 1024
E = 64
D = 64
LT = L // 128
SC = S // 128

_CACHE = {}


def _apply_tile_drain_patch():
    import concourse.tile as tile
    import concourse.mybir as mybir
    from concourse.vector_clock import ScopedClock

    if getattr(tile.TileContext, "_drain_patched", False):
        return

    def _patched(self, tick_clock, wait_clock):
        nc = self.nc
        collector = nc.sync.nop()
        wait_clock.add_sem_waits(
            collector.ins, ScopedClock({None: tick_clock.global_clock})
        )
        waits = list(collector.ins.sync_info.on_wait)
        if len(waits) > 1:
            collector.ins.sync_info.on_wait = waits[:1]
            for w in waits[1:]:
                extra = nc.sync.nop()
                extra.ins.sync_info = mybir.SyncInfo(on_wait=[w], on_update=[])
        nc.sync.drain()
        nc.all_engine_barrier()
        assert self.sems is not None
        popped = nc._tile_sem_poison_stack.pop()
        assert popped is self._sem_poison
        nc.clear_and_free_semaphores(list(self.sems.allocated().values()))
        nc.all_engine_barrier()

    tile.TileContext._drain_and_barrier = _patched
    tile.TileContext._drain_patched = True


def _split_excess_waits(nc, limit=1):
    """walrus here rejects >limit sync-waits per instruction; move excess
    waits onto same-engine NOPs placed just before the instruction."""
    import concourse.mybir as mybir

    cnt = 0
    for bb in nc.main_func.blocks:
        il = list(bb.instructions)
        out = []
        changed = False
        for ins in il:
            si = getattr(ins, "sync_info", None)
            ow = list(si.on_wait) if (si is not None and si.on_wait) else []
            if len(ow) > limit:
                changed = True
                for w in ow[: len(ow) - limit]:
                    cnt += 1
                    nop = mybir.InstNoOp(name=f"WSPLIT-{cnt}", ins=[], outs=[])
                    nop.engine = ins.engine
                    nop.sync_info = mybir.SyncInfo(on_wait=[w], on_update=[])
                    nc.register_instruction(nop, overwrite=True)
                    out.append(nop)
                si.on_wait = ow[len(ow) - limit :]
            out.append(ins)
        if changed:
            bb.instructions = out
    return cnt


def build_program(n_pairs=NP_PAIRS):
    import concourse.bass as bass
    import concourse.mybir as mybir
    import concourse.tile as tile
    from concourse.bass import ts, ds

    _apply_tile_drain_patch()

    f32 = mybir.dt.float32
    bf16 = mybir.dt.bfloat16
    f16 = mybir.dt.float16
    nc = bass.Bass()

    # bf16 stacked score operands
    qhh_d = nc.dram_tensor("qhh", [n_pairs, 128, L], bf16, kind="ExternalInput")
    qll_d = nc.dram_tensor("qll", [n_pairs, 128, L], bf16, kind="ExternalInput")
    khh_d = nc.dram_tensor("khh", [n_pairs, 128, S], bf16, kind="ExternalInput")
    kst_d = nc.dram_tensor("kst", [n_pairs, 128, S], bf16, kind="ExternalInput")
    kh66_d = nc.dram_tensor("kh66", [n_pairs, 66, S], bf16, kind="ExternalInput")
    qlt_d = nc.dram_tensor("qlt", [n_pairs, 66, L], bf16, kind="ExternalInput")
    vh_d = nc.dram_tensor("vh", [n_pairs, S, D], f16, kind="ExternalInput")
    vl_d = nc.dram_tensor("vl", [n_pairs, S, D], f16, kind="ExternalInput")
    recip_d = nc.dram_tensor("recip", [128, 128], f32, kind="ExternalInput")
    nrecip_d = nc.dram_tensor("nrecip", [128, 128], f32, kind="ExternalInput")
    # output stored transposed per pair: [D, L]; host untransposes
    out = nc.dram_tensor("out", [n_pairs, D, L], f32, kind="ExternalOutput")

    with tile.TileContext(nc) as tc:
        with (
            tc.tile_pool(name="singles", bufs=1) as singles,
            tc.tile_pool(name="weights", bufs=4) as weights,
            tc.tile_pool(name="vt_pool", bufs=3) as vt_pool,
            tc.tile_pool(name="zs_pool", bufs=8) as zs_pool,
            tc.tile_pool(name="at_pool", bufs=8) as at_pool,
            tc.tile_pool(name="cand_pool", bufs=6) as cand_pool,
            tc.tile_pool(name="tau_pool", bufs=2) as tau_pool,
            tc.tile_pool(name="o_pool", bufs=2) as o_pool,
            tc.tile_pool(name="zpsum", bufs=2, space="PSUM") as zpsum,
            tc.tile_pool(name="ztpsum", bufs=2, space="PSUM") as ztpsum,
            tc.tile_pool(name="otpsum", bufs=1, space="PSUM") as otpsum,
        ):
            recipt = singles.tile([128, 128], f32)
            nrecipt = singles.tile([128, 128], f32)
            nc.sync.dma_start(out=recipt[:], in_=recip_d[:])
            nc.sync.dma_start(out=nrecipt[:], in_=nrecip_d[:])
            recip_v = recipt.rearrange("p (s r) -> p s r", r=16)
            nrecip_v = nrecipt.rearrange("p (s r) -> p s r", r=16)

            add = mybir.AluOpType.add
            mult = mybir.AluOpType.mult
            sub = mybir.AluOpType.subtract

            for g in range(n_pairs):
                qhh = weights.tile([128, L], bf16, tag="qhh")
                qll = weights.tile([128, L], bf16, tag="qll")
                khh = weights.tile([128, S], bf16, tag="khh")
                kst = weights.tile([128, S], bf16, tag="kst")
                kh66 = weights.tile([66, S], bf16, tag="kh66")
                qlt = weights.tile([66, L], bf16, tag="qlt")
                vh = vt_pool.tile([128, SC, D], f16, tag="vh")
                vl = vt_pool.tile([128, SC, D], f16, tag="vl")
                nc.sync.dma_start(out=qhh[:], in_=qhh_d[g])
                nc.sync.dma_start(out=qll[:], in_=qll_d[g])
                nc.sync.dma_start(out=khh[:], in_=khh_d[g])
                nc.sync.dma_start(out=kst[:], in_=kst_d[g])
                nc.sync.dma_start(out=kh66[:], in_=kh66_d[g])
                nc.sync.dma_start(out=qlt[:, :], in_=qlt_d[g])
                nc.sync.dma_start(
                    out=vh[:], in_=vh_d[g].rearrange("(c p) d -> p c d", p=128)
                )
                nc.sync.dma_start(
                    out=vl[:], in_=vl_d[g].rearrange("(c p) d -> p c d", p=128)
                )

                csA = tau_pool.tile([128, LT, 24], f32, tag="csA")
                csB = tau_pool.tile([128, LT, 24], f32, tag="csB")
                negtau = tau_pool.tile([128, LT], f32, tag="negtau")
                nthl = tau_pool.tile([128, 32], bf16, tag="nthl")
                wthl = tau_pool.tile([128, 32], bf16, tag="wthl")
                nc.gpsimd.memset(csA[:, :, 0:8], 0.0)
                nc.gpsimd.memset(csB[:, :, 0:8], 0.0)
                nc.gpsimd.memset(nthl[:, 2 * LT : 32], 0.0)

                # --- z pass (3-term bf16 split) + candidates ---
                # z ~= [Qh;Qh]^T kst  +  Ql^T Kh   (drop Ql Kl ~ 2^-18)
                # the Ql Kh term runs as row-packed K=64 matmuls: two
                # l-tiles concurrently in array row-groups 0 and 64.
                zps = {}
                for t in range(LT):
                    zp = zpsum.tile([128, S], f32, tag="scores")
                    zps[t] = zp
                    for h in range(2):
                        nc.tensor.matmul(
                            zp[:, ds(512 * h, 512)],
                            qhh[:, ts(t, 128)],
                            kst[:, ds(512 * h, 512)],
                            start=True,
                            stop=False,
                        )
                    if t % 2 == 1:
                        for h in range(2):
                            nc.tensor.matmul(
                                zps[t - 1][:, ds(512 * h, 512)],
                                qll[0:64, ts(t - 1, 128)],
                                khh[0:64, ds(512 * h, 512)],
                                start=False,
                                stop=True,
                            )
                            nc.tensor.matmul(
                                zps[t][:, ds(512 * h, 512)],
                                qll[64:128, ts(t, 128)],
                                khh[64:128, ds(512 * h, 512)],
                                start=False,
                                stop=True,
                            )
                    if t % 2 == 0:
                        continue
                    for tt in (t - 1, t):
                        zp = zps.pop(tt)
                        zs = zs_pool.tile([128, S], f32, tag="zs")
                        nc.scalar.copy(zs[:], zp[:])
                        cand = cand_pool.tile([128, 64], f32, tag="cand")
                        for c in range(SC):
                            nc.vector.max(cand[:, ts(c, 8)], zs[:, ts(c, 128)])
                        cand2 = cand_pool.tile([128, 64], f32, tag="cand2")
                        nc.vector.max(csA[:, tt, 8:16], cand[:])
                        nc.vector.match_replace(
                            cand2[:], csA[:, tt, 8:16], cand[:], -1e30
                        )
                        nc.vector.max(csA[:, tt, 16:24], cand2[:])

                # --- tau (cumsum-16 + max-identity) on gpsimd; DVE is busy ---
                TT = nc.gpsimd.tensor_tensor
                TT(csB[:, :, 8:24], csA[:, :, 8:24], csA[:, :, 7:23], op=add)
                TT(csA[:, :, 8:24], csB[:, :, 8:24], csB[:, :, 6:22], op=add)
                TT(csB[:, :, 8:24], csA[:, :, 8:24], csA[:, :, 4:20], op=add)
                TT(csA[:, :, 8:24], csB[:, :, 8:24], csB[:, :, 0:16], op=add)
                mbuf = tau_pool.tile([128, LT, 16], f32, tag="mbuf")
                stat = tau_pool.tile([128, LT, 16], f32, tag="stat")
                TT(mbuf[:], csA[:, :, 8:24], nrecip_v[:], op=mult)
                TT(stat[:], mbuf[:], recip_v[:], op=add)
                TT = nc.vector.tensor_tensor
                nc.vector.tensor_reduce(
                    negtau[:],
                    stat[:],
                    axis=mybir.AxisListType.X,
                    op=mybir.AluOpType.min,
                )
                # bf16 hi/lo split of -tau: nthl cols 0:8 = -tau_h, 8:16 = -tau_l
                nc.vector.tensor_copy(nthl[:, 0:LT], negtau[:])
                TT(nthl[:, LT : 2 * LT], negtau[:], nthl[:, 0:LT], op=sub)
                # scatter into qlt rows 64/65 via 32x32 stream transpose:
                # wthl[32P+a, b] = nthl[32P+b, a]
                nc.vector.transpose(wthl[:], nthl[:])
                q64 = qlt[64:65, :].rearrange("o (t Pq b) -> o Pq t b", Pq=4, b=32)
                q65 = qlt[65:66, :].rearrange("o (t Pq b) -> o Pq t b", Pq=4, b=32)
                for P in range(4):
                    nc.sync.dma_start(
                        out=q64[:, P], in_=wthl[ds(32 * P, LT), :]
                    )
                    nc.sync.dma_start(
                        out=q65[:, P], in_=wthl[ds(32 * P + LT, LT), :]
                    )

                # --- z^T - tau (bf16 split + K=66 bias), relu->fp16, AV ---
                ot = otpsum.tile([64, L], f32, tag="ot")
                for c in range(SC):
                    ats = []
                    for h in range(2):
                        ztp = ztpsum.tile([128, 512], f32, tag="zt")
                        nc.tensor.matmul(
                            ztp[:],
                            kst[:, ts(c, 128)],
                            qhh[:, ds(512 * h, 512)],
                            start=True,
                            stop=False,
                        )
                        nc.tensor.matmul(
                            ztp[:],
                            kh66[:, ts(c, 128)],
                            qlt[:, ds(512 * h, 512)],
                            start=False,
                            stop=True,
                        )
                        at = at_pool.tile([128, 512], f16, tag="at")
                        if c >= 4 and h == 1:
                            nc.vector.tensor_scalar_max(at[:], ztp[:], 0.0)
                        else:
                            nc.scalar.activation(
                                at[:], ztp[:], mybir.ActivationFunctionType.Relu
                            )
                        ats.append(at)
                    for h in range(2):
                        nc.tensor.matmul(
                            ot[:, ds(512 * h, 512)],
                            vh[:, c, :],
                            ats[h][:],
                            start=(c == 0),
                            stop=False,
                        )
                        nc.tensor.matmul(
                            ot[:, ds(512 * h, 512)],
                            vl[:, c, :],
                            ats[h][:],
                            start=False,
                            stop=(c == SC - 1),
                        )

                # output written transposed [D, L]; contiguous DMA
                ots = o_pool.tile([64, L], f32, tag="ots")
                nc.scalar.copy(ots[:], ot[:])
                nc.sync.dma_start(out=out[g], in_=ots[:])

    n_split = _split_excess_waits(nc)
    if n_split:
        import sys

        print(f"[kernel] split {n_split} excess sync-waits onto NOPs", file=sys.stderr)
    return nc


def _host_prep(queries, keys, values):
    """Build the 8 per-core input maps from full inputs."""
    import ml_dtypes

    bf16 = ml_dtypes.bfloat16
    B, Lq, H, Ee = queries.shape
    scale = np.float32(1.0 / np.sqrt(np.float32(Ee)))
    jj = np.arange(128, dtype=np.float32)
    recip = np.broadcast_to(1.0 / ((jj % 16) + 1.0), (128, 128)).astype(np.float32)
    nrecip = (-recip).astype(np.float32)

    in_maps = []
    for b in range(B):
        qt = (queries[b].astype(np.float32) * scale).transpose(1, 2, 0)  # [H,E,L]
        kt = keys[b].astype(np.float32).transpose(1, 2, 0)  # [H,E,S]
        vv = values[b].astype(np.float32).transpose(1, 0, 2)  # [H,S,D]

        qh = qt.astype(bf16)
        ql = (qt - qh.astype(np.float32)).astype(bf16)
        kh = kt.astype(bf16)
        kl = (kt - kh.astype(np.float32)).astype(bf16)

        qhh = np.concatenate([qh, qh], axis=1)  # [H,128,L]
        qll = np.concatenate([ql, ql], axis=1)
        khh = np.concatenate([kh, kh], axis=1)
        kst = np.concatenate([kh, kl], axis=1)
        ones_row = np.ones((H, 2, S), dtype=bf16)
        kh66 = np.concatenate([kh, ones_row], axis=1)  # [H,66,S]
        qlt = np.concatenate([ql, np.zeros((H, 2, Lq), dtype=bf16)], axis=1)

        vhf = vv.astype(np.float16)
        vlf = (vv - vhf.astype(np.float32)).astype(np.float16)

        in_maps.append(
            {
                "qhh": np.ascontiguousarray(qhh),
                "qll": np.ascontiguousarray(qll),
                "khh": np.ascontiguousarray(khh),
                "kst": np.ascontiguousarray(kst),
                "kh66": np.ascontiguousarray(kh66),
                "qlt": np.ascontiguousarray(qlt),
                "vh": np.ascontiguousarray(vhf),
                "vl": np.ascontiguousarray(vlf),
                "recip": recip.copy(),
                "nrecip": nrecip.copy(),
            }
        )
    return in_maps


def kernel(queries, keys, values):
    from concourse.bass_utils import run_bass_kernel_spmd

    queries = np.asarray(queries)
    keys = np.asarray(keys)
    values = np.asarray(values)
    B, Lq, H, _ = queries.shape

    if "nc" not in _CACHE:
        _CACHE["nc"] = build_program(NP_PAIRS)
    nc = _CACHE["nc"]

    in_maps = _host_prep(queries, keys, values)
    res = run_bass_kernel_spmd(nc, in_maps, list(range(N_CORES)))

    outp = np.empty((B, Lq, H, D), dtype=np.float32)
    for b in range(B):
        o = res.results[b]["out"]  # [H, D, L]
        outp[b] = o.transpose(2, 0, 1)
    return outp
